# revision 1
# baseline (speedup 1.0000x reference)
"""Chamfer-style point loss (nn_PointLoss) on 8 Trainium2 NeuronCores.

Math (reference): reflect points across plane n.x+d=0; half1 = reflected
points (valid where s=p.n+d < 0, mask m1), half2 = original points (mask
m2 = ~m1). D[i,j] = ||half1[i]-half2[j]||^2. Output scalar =
50*(sum_j min_i(D) m2_j / c2 + sum_i min_j(D) m1_i / c1).

v3 device formulation: F[i,j] = r1'[i] + r2'[j] - 2*a_i.b_j with penalty
P=2^14 on masked-out rows/cols. Row and column operand prep is merged
into one (128,36)-wide pass (cols appended as 4 extra columns) using
region-constant tiles; hi/lo bf16 splits write a single composite that is
scattered into K-major DRAM images with rearranged-destination DMAs
spread over the SP/Activation/Pool queues. One K=16 bf16 matmul per
(128,512) tile; tiles negated into fp16 so mins become maxes. Row-min
partials combined via one AllReduce(max) on a (128,33) f32 payload.

Sharding: half2 (column) axis split 8 ways, 512 cols/core; every core
holds all rows.
"""

import os
import sys

import numpy as np

for _p in ("/opt/trn_rl_repo", "/root/.axon_site/_ro/trn_rl_repo"):
    if os.path.isdir(_p) and _p not in sys.path:
        sys.path.insert(0, _p)

import concourse.bacc as bacc
import concourse.bass_isa as bass_isa
import concourse.tile as tile
from concourse import mybir
from concourse.bass_utils import run_bass_kernel_spmd

FP = mybir.dt.float32
BF = mybir.dt.bfloat16
HF = mybir.dt.float16
AX = mybir.AxisListType
OP = mybir.AluOpType

N = 4096
NCORES = 8
QT = 32            # row q-slots (q-inner layout: [p,q] is point 32p+q)
QC = QT // NCORES  # 4 col slots per partition (512 columns/core)
W = QT + QC        # merged row+col working width
PEN = float(2**14)  # keeps penalized F finite in fp16
BIG = 1.0e30
CMINIT = -60000.0


def _emit(tc, out_ap, norm_ap, pa_ap, oh_ap):
    nc = tc.nc

    psf = tc.alloc_tile_pool(name="psf", bufs=2, space="PSUM")
    pss = tc.alloc_tile_pool(name="pss", bufs=2, space="PSUM")
    per = tc.alloc_tile_pool(name="per", bufs=1)
    fsp = tc.alloc_tile_pool(name="fsp", bufs=3)
    drm = tc.alloc_tile_pool(name="drm", bufs=1, space="DRAM")

    def _t(shape, name, dt=FP):
        return per.tile(shape, dt, name=name)

    # ---- inputs to SBUF (split across both hwdge queues)
    norm_sb = _t([1, 4], "norm_sb")
    nc.sync.dma_start(norm_sb[:], norm_ap[:])
    PA = _t([128, W, 3], "PA")
    nc.sync.dma_start(PA[:], pa_ap[:])
    OH = _t([128, 1], "OH")
    nc.scalar.dma_start(OH[:], oh_ap[:])

    ones_c = _t([128, 1], "ones_c")
    nc.gpsimd.memset(ones_c[:], 1.0)
    ONES64 = _t([128, 64], "ONES64", BF)
    nc.gpsimd.memset(ONES64[:], 1.0)

    # region-constant tiles: first QT cols = row region, last QC = col region
    BETA = _t([128, W], "BETA")
    nc.gpsimd.memset(BETA[:, 0:QT], 1.0)
    nc.gpsimd.memset(BETA[:, QT:W], -2.0)
    SGNP = _t([128, W], "SGNP")
    nc.gpsimd.memset(SGNP[:, 0:QT], -PEN)
    nc.gpsimd.memset(SGNP[:, QT:W], PEN)
    ROWP = _t([128, W], "ROWP")
    nc.gpsimd.memset(ROWP[:, 0:QT], PEN)
    nc.gpsimd.memset(ROWP[:, QT:W], 0.0)
    RMASK = _t([128, W], "RMASK")
    nc.gpsimd.memset(RMASK[:, 0:QT], 1.0)
    nc.gpsimd.memset(RMASK[:, QT:W], 0.0)

    # ---- norm broadcast + plane constants
    NB = _t([128, 4], "NB")
    nc.gpsimd.partition_broadcast(NB[:], norm_sb[:], channels=128)
    nsq = _t([128, 4], "nsq")
    nc.vector.tensor_tensor(nsq[:], NB[:], NB[:], op=OP.mult)
    snn = _t([128, 1], "snn")
    nc.vector.tensor_reduce(snn[:], nsq[:, 0:3], axis=AX.X, op=OP.add)
    inv_nn = _t([128, 1], "inv_nn")
    nc.vector.reciprocal(inv_nn[:], snn[:])
    ninv2 = _t([128, 1], "ninv2")
    nc.scalar.mul(ninv2[:], inv_nn[:], -2.0)
    c4d = _t([128, 1], "c4d")
    nc.vector.tensor_tensor(c4d[:], NB[:, 3:4], inv_nn[:], op=OP.mult)
    nc.scalar.mul(c4d[:], c4d[:], 4.0)
    NINV2R = _t([128, W], "NINV2R")
    nc.vector.tensor_scalar(NINV2R[:], RMASK[:], ninv2[:], None, op0=OP.mult)
    C4DR = _t([128, W], "C4DR")
    nc.scalar.mul(C4DR[:], RMASK[:], c4d[:])

    # ---- merged plane eval: s = p.n + d, m1 = (s<0)
    s_all = _t([128, W], "s_all")
    t1_ = _t([128, W], "t1_")
    nc.scalar.mul(s_all[:], PA[:, :, 0], NB[:, 0:1])
    nc.scalar.mul(t1_[:], PA[:, :, 1], NB[:, 1:2])
    nc.vector.tensor_tensor(s_all[:], s_all[:], t1_[:], op=OP.add)
    nc.scalar.mul(t1_[:], PA[:, :, 2], NB[:, 2:3])
    nc.vector.tensor_tensor(s_all[:], s_all[:], t1_[:], op=OP.add)
    nc.vector.tensor_scalar_add(s_all[:], s_all[:], NB[:, 3:4])
    M1 = _t([128, W], "M1")
    nc.vector.tensor_scalar(M1[:], s_all[:], 0.0, None, op0=OP.is_lt)

    # ---- operand vectors V = alpha*n + beta.p (rows: reflect, cols: -2p)
    alpha = _t([128, W], "alpha")
    nc.vector.tensor_tensor(alpha[:], s_all[:], NINV2R[:], op=OP.mult)
    V = []
    for c in range(3):
        tv = _t([128, W], f"tv{c}")
        nc.scalar.mul(tv[:], alpha[:], NB[:, c : c + 1])
        tb2 = _t([128, W], f"tb2{c}")
        nc.gpsimd.tensor_tensor(tb2[:], BETA[:], PA[:, :, c], op=OP.mult)
        v = _t([128, W], f"v{c}")
        nc.vector.tensor_tensor(v[:], tv[:], tb2[:], op=OP.add)
        V.append(v)

    # ---- rr = |p|^2 + (4d/nn)*s*is_row + penalty
    pp = _t([128, W], "pp")
    nc.vector.tensor_tensor(pp[:], PA[:, :, 0], PA[:, :, 0], op=OP.mult)
    q1 = _t([128, W], "q1")
    nc.gpsimd.tensor_tensor(q1[:], PA[:, :, 1], PA[:, :, 1], op=OP.mult)
    nc.vector.tensor_tensor(pp[:], pp[:], q1[:], op=OP.add)
    q2 = _t([128, W], "q2")
    nc.gpsimd.tensor_tensor(q2[:], PA[:, :, 2], PA[:, :, 2], op=OP.mult)
    nc.vector.tensor_tensor(pp[:], pp[:], q2[:], op=OP.add)
    t3 = _t([128, W], "t3")
    nc.vector.tensor_tensor(t3[:], s_all[:], C4DR[:], op=OP.mult)
    t4 = _t([128, W], "t4")
    nc.gpsimd.tensor_tensor(t4[:], M1[:], SGNP[:], op=OP.mult)
    nc.gpsimd.tensor_tensor(t4[:], t4[:], ROWP[:], op=OP.add)
    rr = _t([128, W], "rr")
    nc.vector.tensor_tensor(rr[:], pp[:], t3[:], op=OP.add)
    nc.vector.tensor_tensor(rr[:], rr[:], t4[:], op=OP.add)

    # ---- bf16 hi/lo splits into one composite: slots [Vh0-2, Vl0-2, rrh, rrl]
    ACOMP = _t([128, 8, W], "ACOMP", BF)

    def split_into(src, hs, ls, name):
        nc.scalar.copy(ACOMP[:, hs, :], src[:])
        hf = _t([128, W], f"hf_{name}")
        nc.vector.tensor_scalar(hf[:], ACOMP[:, hs, :], 1.0, None, op0=OP.mult)
        lr = _t([128, W], f"lr_{name}")
        nc.gpsimd.tensor_tensor(lr[:], src[:], hf[:], op=OP.subtract)
        nc.scalar.copy(ACOMP[:, ls, :], lr[:])

    for c in range(3):
        split_into(V[c], c, 3 + c, f"v{c}")
    split_into(rr, 6, 7, "rr")

    # ---- K-major operand images via DRAM round trip (rearranged dests)
    stgA = drm.tile([16, 128, QT], BF, name="stgA")
    stgB = drm.tile([16, 128, QC], BF, name="stgB")

    AHI = ACOMP[:, 0:3, 0:QT]
    ALO = ACOMP[:, 3:6, 0:QT]
    ARR = ACOMP[:, 6:8, 0:QT]
    nc.sync.dma_start(stgA[0:3, :, :].rearrange("k i m -> i k m"), AHI)
    nc.scalar.dma_start(stgA[3:6, :, :].rearrange("k i m -> i k m"), AHI)
    nc.sync.dma_start(stgA[6:9, :, :].rearrange("k i m -> i k m"), ALO)
    nc.scalar.dma_start(stgA[9:12, :, :].rearrange("k i m -> i k m"), ALO)
    nc.sync.dma_start(stgA[12:14, :, :].rearrange("k i m -> i k m"), ARR)
    nc.gpsimd.dma_start(stgA[14:16, :, :], ONES64[:])

    BHI = ACOMP[:, 0:3, QT:W]
    BLO = ACOMP[:, 3:6, QT:W]
    BRR = ACOMP[:, 6:8, QT:W]
    nc.scalar.dma_start(stgB[0:3, :, :].rearrange("k p q -> p k q"), BHI)
    nc.gpsimd.dma_start(stgB[3:6, :, :].rearrange("k p q -> p k q"), BLO)
    nc.gpsimd.dma_start(stgB[6:9, :, :].rearrange("k p q -> p k q"), BHI)
    nc.gpsimd.dma_start(stgB[9:12, :, :].rearrange("k p q -> p k q"), BLO)
    nc.gpsimd.dma_start(stgB[12:14, :, :], ONES64[:, 0:8])
    nc.scalar.dma_start(stgB[14:16, :, :].rearrange("k p q -> p k q"), BRR)

    TA = _t([16, 128, QT], "TA", BF)
    nc.sync.dma_start(TA[:], stgA[:])
    TB = _t([16, 128, QC], "TB", BF)
    nc.scalar.dma_start(TB[:], stgB[:])

    # ---- masks: M1 tile-layout via DRAM reshuffle; m2 as (1,512) row
    stgQ = drm.tile([QT, 128], FP, name="stgQ")
    nc.gpsimd.dma_start(stgQ[:], M1[:, 0:QT])
    M1t = _t([128, QT], "M1t")
    nc.gpsimd.dma_start(M1t[:], stgQ[:].rearrange("g p -> p g"))
    M2CB = _t([128, QC], "M2CB")
    nc.vector.tensor_scalar(M2CB[:], M1[:, QT:W], -1.0, 1.0, op0=OP.mult, op1=OP.add)
    stgM = drm.tile([128, QC], FP, name="stgM")
    nc.gpsimd.dma_start(stgM[:], M2CB[:])
    M2row = _t([1, 512], "M2row")
    nc.gpsimd.dma_start(M2row[:], stgM[:])

    # ---- c1/c2 + reciprocals precomputed before the collective
    c1row = _t([128, 1], "c1row")
    nc.vector.tensor_reduce(c1row[:], M1[:, 0:QT], axis=AX.X, op=OP.add)
    c1_ps = pss.tile([1, 1], FP, tag="ps")
    nc.tensor.matmul(c1_ps[:], c1row[:], ones_c[:], start=True, stop=True)
    c1 = _t([1, 1], "c1")
    nc.scalar.copy(c1[:], c1_ps[:])
    c2 = _t([1, 1], "c2")
    nc.vector.tensor_scalar(c2[:], c1[:], -1.0, float(N), op0=OP.mult, op1=OP.add)
    nc.vector.tensor_scalar_max(c1[:], c1[:], 1.0)
    nc.vector.tensor_scalar_max(c2[:], c2[:], 1.0)
    rc1 = _t([1, 1], "rc1")
    nc.vector.reciprocal(rc1[:], c1[:])
    rc2 = _t([1, 1], "rc2")
    nc.vector.reciprocal(rc2[:], c2[:])

    # ---- main loop: FS = -(F tile) in fp16; row-max partials + col-max acc
    CM = _t([128, 512], "CM", HF)
    nc.gpsimd.memset(CM[:], CMINIT)
    D2 = _t([128, QT], "D2", HF)

    for m in range(QT):
        fps = psf.tile([128, 512], FP, name="fps")
        nc.tensor.matmul(
            fps[:], TA[:, 4 * m : 4 * (m + 1), :], TB[:], start=True, stop=True
        )
        FS = fsp.tile([128, 512], HF, name="FS")
        nc.scalar.mul(FS[:], fps[:], -1.0)
        nc.vector.tensor_reduce(D2[:, m : m + 1], FS[:], axis=AX.X, op=OP.max)
        nc.vector.tensor_tensor(CM[:], CM[:], FS[:], op=OP.max)

    # ---- columns: d1 = max over partitions, then masked sum s1
    D1B = _t([128, 512], "D1B")
    nc.gpsimd.partition_all_reduce(D1B[:], CM[:], 128, bass_isa.ReduceOp.max)
    w1 = _t([1, 512], "w1")
    nc.vector.tensor_tensor(w1[:], D1B[0:1, :], M2row[:], op=OP.mult)
    s1 = _t([1, 1], "s1")
    nc.vector.tensor_reduce(s1[:], w1[:], axis=AX.X, op=OP.add)

    # encode s1 into partition slot column: slot[core_id] = s1, else -BIG
    s1b = _t([128, 1], "s1b")
    nc.gpsimd.partition_broadcast(s1b[:], s1[:], channels=128)
    slot = _t([128, 1], "slot")
    nc.vector.tensor_tensor(slot[:], s1b[:], OH[:], op=OP.mult)
    bm = _t([128, 1], "bm")
    nc.vector.tensor_scalar(bm[:], OH[:], BIG, -BIG, op0=OP.mult, op1=OP.add)
    nc.vector.tensor_tensor(slot[:], slot[:], bm[:], op=OP.add)

    D2f = _t([128, QT], "D2f")
    nc.scalar.copy(D2f[:], D2[:])

    # ---- AllReduce(max) of [D2 | slot] over all 8 cores
    pay = drm.tile([128, QT + 1], FP, name="pay")
    pay2 = drm.tile([128, QT + 1], FP, name="pay2")
    nc.gpsimd.dma_start(pay[:, 0:QT], D2f[:])
    nc.gpsimd.dma_start(pay[:, QT : QT + 1], slot[:])
    nc.gpsimd.collective_compute(
        "AllReduce",
        OP.max,
        replica_groups=[list(range(NCORES))],
        ins=[pay.opt()],
        outs=[pay2.opt()],
    )
    G2 = _t([128, QT], "G2")
    nc.gpsimd.dma_start(G2[:], pay2[:, 0:QT])
    slots = _t([128, 1], "slots")
    nc.gpsimd.dma_start(slots[:], pay2[:, QT : QT + 1])

    # ---- finish: s2 = sum(G2*m1t), sum slots, combine with rc1/rc2
    w2 = _t([128, QT], "w2")
    nc.vector.tensor_tensor(w2[:], G2[:], M1t[:], op=OP.mult)
    w2s = _t([128, 1], "w2s")
    nc.vector.tensor_reduce(w2s[:], w2[:], axis=AX.X, op=OP.add)
    s2_ps = pss.tile([1, 1], FP, tag="ps")
    nc.tensor.matmul(s2_ps[:], w2s[:], ones_c[:], start=True, stop=True)

    sa_ps = pss.tile([1, 1], FP, tag="ps")
    nc.tensor.matmul(
        sa_ps[:], slots[0:NCORES, :], ones_c[0:NCORES, :], start=True, stop=True
    )

    s2 = _t([1, 1], "s2")
    nc.scalar.copy(s2[:], s2_ps[:])
    sum_s1 = _t([1, 1], "sum_s1")
    nc.scalar.copy(sum_s1[:], sa_ps[:])
    av2 = _t([1, 1], "av2")
    nc.vector.tensor_tensor(av2[:], s2[:], rc1[:], op=OP.mult)
    av1 = _t([1, 1], "av1")
    nc.vector.tensor_tensor(av1[:], sum_s1[:], rc2[:], op=OP.mult)
    res = _t([1, 1], "res")
    nc.vector.tensor_tensor(res[:], av1[:], av2[:], op=OP.add)
    nc.scalar.mul(res[:], res[:], -50.0)
    nc.sync.dma_start(out_ap[:], res[:])

    for p in (psf, pss, per, fsp, drm):
        p.seal()


_NC = None


def build():
    global _NC
    if _NC is not None:
        return _NC
    nc = bacc.Bacc(
        "TRN2", target_bir_lowering=False, debug=False, num_devices=NCORES
    )
    norm_ap = nc.dram_tensor("norm4", [1, 4], FP, kind="ExternalInput").ap()
    pa_ap = nc.dram_tensor("pa", [128, W, 3], FP, kind="ExternalInput").ap()
    oh_ap = nc.dram_tensor("oh", [128, 1], FP, kind="ExternalInput").ap()
    out_ap = nc.dram_tensor("out", [1, 1], FP, kind="ExternalOutput").ap()
    with tile.TileContext(nc) as tc:
        _emit(tc, out_ap, norm_ap, pa_ap, oh_ap)
    nc.compile()
    _NC = nc
    return nc


def make_in_maps(norm, points):
    norm = np.ascontiguousarray(norm, dtype=np.float32)
    pts = np.ascontiguousarray(points, dtype=np.float32)
    PTq = pts.reshape(128, QT, 3)
    maps = []
    for c in range(NCORES):
        oh = np.zeros((128, 1), np.float32)
        oh[c, 0] = 1.0
        cb = pts[512 * c : 512 * (c + 1)].reshape(128, QC, 3)
        pa = np.ascontiguousarray(np.concatenate([PTq, cb], axis=1))
        maps.append({"norm4": norm, "pa": pa, "oh": oh})
    return maps


LAST_RESULTS = None


def kernel(norm, points):
    global LAST_RESULTS
    nc = build()
    maps = make_in_maps(norm, points)
    trace = bool(os.environ.get("KERNEL_TRACE"))
    LAST_RESULTS = run_bass_kernel_spmd(
        nc, maps, list(range(NCORES)), trace=trace
    )
    out = np.asarray(LAST_RESULTS.results[0]["out"], dtype=np.float32)
    return out.reshape(())



# revision 16
# speedup vs baseline: 1.3458x; 1.3458x over previous
"""Chamfer-style point loss (nn_PointLoss) on 8 Trainium2 NeuronCores.

Math (reference): reflect points across plane n.x+d=0; half1 = reflected
points (valid where s=p.n+d < 0, mask m1), half2 = original points (mask
m2 = ~m1). D[i,j] = ||half1[i]-half2[j]||^2. Output scalar =
50*(sum_j min_i(D) m2_j / c2 + sum_i min_j(D) m1_i / c1).

v4 device formulation: F[i,j] = rr1[i] + rr2[j] + a_i.(-2 b_j) with
penalty P=2^14 added to masked-out rows/cols, computed as one K=16 bf16
hi/lo matmul per (128,512) tile and min-reductions (no negation trick).
Points are laid out partition-inner (pt = 128*j + p) so the K-major
operand images are built with PE transposes of the on-chip composites —
no DRAM round trips, no scatter DMAs. Column mins finish with 4 more PE
transposes + free-axis reductions (no partition_all_reduce). Row-min
partials + the slot-encoded per-core column sum ride one bf16
AllReduce(min) into a Shared DRAM output; a tiny dummy collective at
kernel start warms up the CC stream/barrier under the compute.

Sharding: half2 (column) axis split 8 ways, 512 cols/core; every core
holds all rows.
"""

import os
import sys

import numpy as np

for _p in ("/opt/trn_rl_repo", "/root/.axon_site/_ro/trn_rl_repo"):
    if os.path.isdir(_p) and _p not in sys.path:
        sys.path.insert(0, _p)

import concourse.bacc as bacc
import concourse.tile as tile
from concourse import masks, mybir
from concourse.bass_utils import run_bass_kernel_spmd

FP = mybir.dt.float32
BF = mybir.dt.bfloat16
AX = mybir.AxisListType
OP = mybir.AluOpType

N = 4096
NCORES = 8
QR = 32            # row chunks of 128 points (pt = 128*j + p)
QC = 4             # col chunks per core (512 cols/core)
W = QR + QC        # merged row+col working width
PEN = float(2**14)
SENT = 60000.0     # min-identity sentinel, bf16-safe


def _emit(tc, out_ap, norm_ap, pa_ap, oh_ap):
    nc = tc.nc

    psf = tc.alloc_tile_pool(name="psf", bufs=3, space="PSUM")
    ptp = tc.alloc_tile_pool(name="ptp", bufs=2, space="PSUM")
    pss = tc.alloc_tile_pool(name="pss", bufs=2, space="PSUM")
    per = tc.alloc_tile_pool(name="per", bufs=1)
    fsp = tc.alloc_tile_pool(name="fsp", bufs=3)
    drm = tc.alloc_tile_pool(name="drm", bufs=1, space="DRAM")

    def _t(shape, name, dt=FP):
        return per.tile(shape, dt, name=name)

    # ---- dummy collective: warm up CC stream + absorb core-launch skew
    zsb = _t([1, 1], "zsb")
    nc.gpsimd.memset(zsb[:], 0.0)
    zdum = drm.tile([1, 1], FP, name="zdum")
    nc.gpsimd.dma_start(zdum[:], zsb[:])
    zdout = drm.tile([1, 1], FP, name="zdout", addr_space="Shared")
    nc.gpsimd.collective_compute(
        "AllReduce",
        OP.add,
        replica_groups=[list(range(NCORES))],
        ins=[zdum.opt()],
        outs=[zdout.opt()],
    )

    # ---- inputs
    norm_sb = _t([1, 4], "norm_sb")
    nc.scalar.dma_start(norm_sb[:], norm_ap[:])
    PA = _t([128, W, 3], "PA")
    nc.sync.dma_start(PA[:], pa_ap[:])
    OH = _t([128, 1], "OH")
    nc.scalar.dma_start(OH[:], oh_ap[:])

    # ---- constants (no input deps; scheduler runs them under the DMAs)
    ident = _t([128, 128], "ident", BF)
    masks.make_identity(nc, ident[:])
    ones_c = _t([128, 1], "ones_c")
    nc.gpsimd.memset(ones_c[:], 1.0)
    CM = _t([128, 512], "CM", BF)
    nc.gpsimd.memset(CM[:], -SENT)
    ACOMP = _t([128, QR, 16], "ACOMP", BF)
    nc.gpsimd.memset(ACOMP[:, :, 14:16], 1.0)
    BCOMP = _t([128, QC, 16], "BCOMP", BF)
    nc.gpsimd.memset(BCOMP[:, :, 12:14], 1.0)

    # ---- norm broadcast + plane constants
    NB = _t([128, 4], "NB")
    nc.gpsimd.partition_broadcast(NB[:], norm_sb[:], channels=128)
    nsq = _t([128, 4], "nsq")
    nc.vector.tensor_tensor(nsq[:], NB[:], NB[:], op=OP.mult)
    snn = _t([128, 1], "snn")
    nc.vector.tensor_reduce(snn[:], nsq[:, 0:3], axis=AX.X, op=OP.add)
    inv_nn = _t([128, 1], "inv_nn")
    nc.vector.reciprocal(inv_nn[:], snn[:])
    ninv2 = _t([128, 1], "ninv2")
    nc.scalar.mul(ninv2[:], inv_nn[:], -2.0)
    c4d = _t([128, 1], "c4d")
    nc.vector.tensor_tensor(c4d[:], NB[:, 3:4], inv_nn[:], op=OP.mult)
    nc.scalar.mul(c4d[:], c4d[:], 4.0)

    # ---- plane eval: s = p.n + d over all 36 chunks; m1 = (s<0)
    s_all = _t([128, W], "s_all")
    t1_ = _t([128, W], "t1_")
    nc.scalar.mul(s_all[:], PA[:, :, 0], NB[:, 0:1])
    nc.scalar.mul(t1_[:], PA[:, :, 1], NB[:, 1:2])
    nc.vector.tensor_tensor(s_all[:], s_all[:], t1_[:], op=OP.add)
    nc.scalar.mul(t1_[:], PA[:, :, 2], NB[:, 2:3])
    nc.vector.tensor_tensor(s_all[:], s_all[:], t1_[:], op=OP.add)
    nc.vector.tensor_scalar_add(s_all[:], s_all[:], NB[:, 3:4])
    M1 = _t([128, W], "M1")
    nc.vector.tensor_scalar(M1[:], s_all[:], 0.0, None, op0=OP.is_lt)

    # ---- operand vectors: rows a = p + alpha*n, cols b' = -2p
    alpha = _t([128, W], "alpha")
    nc.scalar.mul(alpha[:], s_all[:], ninv2[:])
    V3 = _t([128, 3, W], "V3")
    for c in range(3):
        tv = _t([128, QR], f"tv{c}")
        nc.scalar.mul(tv[:], alpha[:, 0:QR], NB[:, c : c + 1])
        nc.vector.tensor_tensor(
            V3[:, c, 0:QR], tv[:], PA[:, 0:QR, c], op=OP.add
        )
        nc.scalar.mul(V3[:, c, QR:W], PA[:, QR:W, c], -2.0)

    # ---- rr = |p|^2 + (4d/nn)*s (rows) + penalties
    pp = _t([128, W], "pp")
    q1 = _t([128, W], "q1")
    q2 = _t([128, W], "q2")
    nc.vector.tensor_tensor(pp[:], PA[:, :, 0], PA[:, :, 0], op=OP.mult)
    nc.gpsimd.tensor_tensor(q1[:], PA[:, :, 1], PA[:, :, 1], op=OP.mult)
    nc.gpsimd.tensor_tensor(q2[:], PA[:, :, 2], PA[:, :, 2], op=OP.mult)
    nc.vector.tensor_tensor(pp[:], pp[:], q1[:], op=OP.add)
    nc.vector.tensor_tensor(pp[:], pp[:], q2[:], op=OP.add)
    t3r = _t([128, QR], "t3r")
    nc.scalar.mul(t3r[:], s_all[:, 0:QR], c4d[:])
    t4r = _t([128, QR], "t4r")
    nc.vector.tensor_scalar(
        t4r[:], M1[:, 0:QR], -PEN, PEN, op0=OP.mult, op1=OP.add
    )
    t4c = _t([128, QC], "t4c")
    nc.scalar.mul(t4c[:], M1[:, QR:W], PEN)
    rr = _t([128, W], "rr")
    nc.vector.tensor_tensor(rr[:, 0:QR], pp[:, 0:QR], t3r[:], op=OP.add)
    nc.vector.tensor_tensor(rr[:, 0:QR], rr[:, 0:QR], t4r[:], op=OP.add)
    nc.gpsimd.tensor_tensor(rr[:, QR:W], pp[:, QR:W], t4c[:], op=OP.add)

    # ---- bf16 hi/lo splits
    VH3 = _t([128, 3, W], "VH3", BF)
    nc.scalar.copy(VH3[:], V3[:])
    vhf = _t([128, 3, W], "vhf")
    nc.vector.tensor_scalar(vhf[:], VH3[:], 1.0, None, op0=OP.mult)
    vlo = _t([128, 3, W], "vlo")
    nc.gpsimd.tensor_tensor(vlo[:], V3[:], vhf[:], op=OP.subtract)
    VL3 = _t([128, 3, W], "VL3", BF)
    nc.scalar.copy(VL3[:], vlo[:])
    RRH = _t([128, W], "RRH", BF)
    nc.scalar.copy(RRH[:], rr[:])
    rhf = _t([128, W], "rhf")
    nc.vector.tensor_scalar(rhf[:], RRH[:], 1.0, None, op0=OP.mult)
    rlo = _t([128, W], "rlo")
    nc.gpsimd.tensor_tensor(rlo[:], rr[:], rhf[:], op=OP.subtract)
    RRL = _t([128, W], "RRL", BF)
    nc.scalar.copy(RRL[:], rlo[:])

    # ---- composite assembly (K slots c-inner, chunk j outer)
    # A slots: [vh vh vl vl rrh rrl 1 1], B slots: [bh bl bh bl 1 1 rrh rrl]
    AHsrc = VH3[:, :, 0:QR].rearrange("p c j -> p j c")
    ALsrc = VL3[:, :, 0:QR].rearrange("p c j -> p j c")
    nc.vector.tensor_copy(ACOMP[:, :, 0:3], AHsrc)
    nc.scalar.copy(ACOMP[:, :, 3:6], AHsrc)
    nc.scalar.copy(ACOMP[:, :, 6:9], ALsrc)
    nc.vector.tensor_copy(ACOMP[:, :, 9:12], ALsrc)
    nc.vector.tensor_copy(ACOMP[:, :, 12:13], RRH[:, 0:QR])
    nc.scalar.copy(ACOMP[:, :, 13:14], RRL[:, 0:QR])

    BHsrc = VH3[:, :, QR:W].rearrange("p c j -> p j c")
    BLsrc = VL3[:, :, QR:W].rearrange("p c j -> p j c")
    nc.vector.tensor_copy(BCOMP[:, :, 0:3], BHsrc)
    nc.vector.tensor_copy(BCOMP[:, :, 3:6], BLsrc)
    nc.scalar.copy(BCOMP[:, :, 6:9], BHsrc)
    nc.vector.tensor_copy(BCOMP[:, :, 9:12], BLsrc)
    nc.vector.tensor_copy(BCOMP[:, :, 14:15], RRH[:, QR:W])
    nc.scalar.copy(BCOMP[:, :, 15:16], RRL[:, QR:W])

    # ---- K-major operand images: PE transposes make point index p the
    # contiguous axis, then a compact DRAM bounce regroups partitions
    # (16*dj+c -> c) with 256B-run descriptors (no scatter DMAs).
    TAT = _t([128, 4, 128], "TAT", BF)
    for g in range(4):
        ptA = ptp.tile([128, 128], BF, tag="tp")
        nc.tensor.transpose(ptA[:], ACOMP[:, 8 * g : 8 * (g + 1), :], ident[:])
        if g % 2 == 0:
            nc.vector.tensor_copy(TAT[:, g, :], ptA[:])
        else:
            nc.scalar.copy(TAT[:, g, :], ptA[:])
    stgA = drm.tile([128, 4, 128], BF, name="stgA")
    nc.sync.dma_start(stgA[:], TAT[:])
    TASB = _t([16, QR, 128], "TASB", BF)
    nc.scalar.dma_start(
        TASB[:].rearrange("c (g dj) p -> c g dj p", dj=8),
        stgA[:].rearrange("(dj c) g p -> c g dj p", c=16),
    )
    TBT = _t([64, 128], "TBT", BF)
    ptB = ptp.tile([128, 128], BF, tag="tp")
    nc.tensor.transpose(ptB[0:64, :], BCOMP[:], ident[:])
    nc.vector.tensor_copy(TBT[:], ptB[0:64, :])
    stgB = drm.tile([64, 128], BF, name="stgB")
    nc.gpsimd.dma_start(stgB[:], TBT[:])
    TBSB = _t([16, QC, 128], "TBSB", BF)
    nc.gpsimd.dma_start(
        TBSB[:], stgB[:].rearrange("(jc c) p -> c jc p", c=16)
    )

    # ---- c1/c2 + reciprocals (before the collective)
    c1row = _t([128, 1], "c1row")
    nc.vector.tensor_reduce(c1row[:], M1[:, 0:QR], axis=AX.X, op=OP.add)
    c1_ps = pss.tile([1, 1], FP, tag="ps")
    nc.tensor.matmul(c1_ps[:], c1row[:], ones_c[:], start=True, stop=True)
    c1 = _t([1, 1], "c1")
    nc.scalar.copy(c1[:], c1_ps[:])
    c2 = _t([1, 1], "c2")
    nc.vector.tensor_scalar(c2[:], c1[:], -1.0, float(N), op0=OP.mult, op1=OP.add)
    nc.vector.tensor_scalar_max(c1[:], c1[:], 1.0)
    nc.vector.tensor_scalar_max(c2[:], c2[:], 1.0)
    rc1 = _t([1, 1], "rc1")
    nc.vector.reciprocal(rc1[:], c1[:])
    rc2 = _t([1, 1], "rc2")
    nc.vector.reciprocal(rc2[:], c2[:])
    M2CB = _t([128, QC], "M2CB")
    nc.vector.tensor_scalar(
        M2CB[:], M1[:, QR:W], -1.0, 1.0, op0=OP.mult, op1=OP.add
    )

    # ---- main loop: one K=16 matmul per (128,512) tile; min reductions
    D2 = _t([128, QR], "D2", BF)
    for m in range(QR):
        g, dj = divmod(m, 8)
        fps = psf.tile([128, 512], FP, tag="mm")
        nc.tensor.matmul(
            fps[:],
            TASB[:, m, :],
            TBSB[:, :, :],
            start=True,
            stop=True,
        )
        FScp = fsp.tile([128, 512], BF, tag="fs")
        nc.scalar.mul(FScp[:], fps[:], -1.0)
        nc.vector.tensor_reduce(D2[:, m : m + 1], FScp[:], axis=AX.X, op=OP.max)
        nc.vector.tensor_tensor(CM[:], CM[:], FScp[:], op=OP.max)

    # ---- columns: transpose CM chunks; d1 = min over rows via free-axis
    d1R = _t([128, QC], "d1R")
    for jc in range(QC):
        ptC = ptp.tile([128, 128], BF, tag="tp")
        nc.tensor.transpose(
            ptC[:], CM[:, 128 * jc : 128 * (jc + 1)], ident[:]
        )
        nc.vector.tensor_reduce(
            d1R[:, jc : jc + 1], ptC[:], axis=AX.X, op=OP.max
        )
    w1 = _t([128, QC], "w1")
    nc.vector.tensor_tensor(w1[:], d1R[:], M2CB[:], op=OP.mult)
    w1s = _t([128, 1], "w1s")
    nc.vector.tensor_reduce(w1s[:], w1[:], axis=AX.X, op=OP.add)
    s1_ps = pss.tile([1, 1], FP, tag="ps")
    nc.tensor.matmul(s1_ps[:], w1s[:], ones_c[:], start=True, stop=True)
    s1 = _t([1, 1], "s1")
    nc.scalar.copy(s1[:], s1_ps[:])

    # encode s1 into partition slot column: slot[core_id] = s1, else SENT
    s1b = _t([128, 1], "s1b")
    nc.gpsimd.partition_broadcast(s1b[:], s1[:], channels=128)
    slott = _t([128, 1], "slott")
    nc.vector.tensor_scalar(
        slott[:], OH[:], SENT, -SENT, op0=OP.mult, op1=OP.add
    )
    slot = _t([128, 1], "slot")
    nc.vector.tensor_tensor(slot[:], s1b[:], OH[:], op=OP.mult)
    nc.vector.tensor_tensor(slot[:], slot[:], slott[:], op=OP.add)

    # ---- AllReduce(min) of bf16 [D2 | slot] over all 8 cores
    PAYS = _t([128, QR + 1], "PAYS", BF)
    nc.vector.tensor_copy(PAYS[:, 0:QR], D2[:])
    nc.scalar.copy(PAYS[:, QR : QR + 1], slot[:])
    pay = drm.tile([128, QR + 1], BF, name="pay")
    pay2 = drm.tile([128, QR + 1], BF, name="pay2", addr_space="Shared")
    nc.sync.dma_start(pay[:], PAYS[:])
    nc.gpsimd.collective_compute(
        "AllReduce",
        OP.max,
        replica_groups=[list(range(NCORES))],
        ins=[pay.opt()],
        outs=[pay2.opt()],
    )
    G2sb = _t([128, QR + 1], "G2sb", BF)
    nc.sync.dma_start(G2sb[:], pay2[:])

    # ---- finish: s2 = sum(G2*m1), slot sum, combine with rc1/rc2
    G2f = _t([128, QR], "G2f")
    nc.scalar.copy(G2f[:], G2sb[:, 0:QR])
    w2 = _t([128, QR], "w2")
    nc.vector.tensor_tensor(w2[:], G2f[:], M1[:, 0:QR], op=OP.mult)
    w2s = _t([128, 1], "w2s")
    nc.vector.tensor_reduce(w2s[:], w2[:], axis=AX.X, op=OP.add)
    s2_ps = pss.tile([1, 1], FP, tag="ps")
    nc.tensor.matmul(s2_ps[:], w2s[:], ones_c[:], start=True, stop=True)
    slotsf = _t([128, 1], "slotsf")
    nc.scalar.copy(slotsf[:], G2sb[:, QR : QR + 1])
    sa_ps = pss.tile([1, 1], FP, tag="ps")
    nc.tensor.matmul(
        sa_ps[:], slotsf[0:NCORES, :], ones_c[0:NCORES, :], start=True, stop=True
    )
    s2 = _t([1, 1], "s2")
    nc.scalar.copy(s2[:], s2_ps[:])
    ssum = _t([1, 1], "ssum")
    nc.scalar.copy(ssum[:], sa_ps[:])
    av2 = _t([1, 1], "av2")
    nc.vector.tensor_tensor(av2[:], s2[:], rc1[:], op=OP.mult)
    av1 = _t([1, 1], "av1")
    nc.vector.tensor_tensor(av1[:], ssum[:], rc2[:], op=OP.mult)
    res = _t([1, 1], "res")
    nc.vector.tensor_tensor(res[:], av1[:], av2[:], op=OP.add)
    nc.scalar.mul(res[:], res[:], -50.0)
    nc.sync.dma_start(out_ap[:], res[:])

    for p in (psf, ptp, pss, per, fsp, drm):
        p.seal()


_NC = None


def build():
    global _NC
    if _NC is not None:
        return _NC
    nc = bacc.Bacc(
        "TRN2", target_bir_lowering=False, debug=False, num_devices=NCORES
    )
    norm_ap = nc.dram_tensor("norm4", [1, 4], FP, kind="ExternalInput").ap()
    pa_ap = nc.dram_tensor("pa", [128, W, 3], FP, kind="ExternalInput").ap()
    oh_ap = nc.dram_tensor("oh", [128, 1], FP, kind="ExternalInput").ap()
    out_ap = nc.dram_tensor("out", [1, 1], FP, kind="ExternalOutput").ap()
    with tile.TileContext(nc) as tc:
        _emit(tc, out_ap, norm_ap, pa_ap, oh_ap)
    nc.compile()
    _NC = nc
    return nc


def make_in_maps(norm, points):
    norm = np.ascontiguousarray(norm, dtype=np.float32)
    pts = np.ascontiguousarray(points, dtype=np.float32)
    # rows: pt = 128*j + p  -> PA[p, j, c]
    PTr = pts.reshape(QR, 128, 3).transpose(1, 0, 2)
    maps = []
    for c in range(NCORES):
        oh = np.zeros((128, 1), np.float32)
        oh[c, 0] = 1.0
        cb = (
            pts[512 * c : 512 * (c + 1)]
            .reshape(QC, 128, 3)
            .transpose(1, 0, 2)
        )
        pa = np.ascontiguousarray(np.concatenate([PTr, cb], axis=1))
        maps.append({"norm4": norm, "pa": pa, "oh": oh})
    return maps


LAST_RESULTS = None


def kernel(norm, points):
    global LAST_RESULTS
    nc = build()
    maps = make_in_maps(norm, points)
    trace = bool(os.environ.get("KERNEL_TRACE"))
    LAST_RESULTS = run_bass_kernel_spmd(
        nc, maps, list(range(NCORES)), trace=trace
    )
    out = np.asarray(LAST_RESULTS.results[0]["out"], dtype=np.float32)
    return out.reshape(())
